# revision 1
# baseline (speedup 1.0000x reference)
"""Sigmoid self-attention Bass kernel for Trainium2, SPMD on 8 cores. v2.

Problem: B=4, S=1024, F=256, H=8
  q = (X @ Wq).reshape(b,s,f,h); k,v likewise (self-attention)
  attn = sigmoid(sqrt(F) * q.kT) per (b,h);  wv = attn @ v
  out = relu(wv_flat @ Wo)

Sharding: data-parallel over (batch, seq-half): core c handles batch c//2,
query rows [half*512, half*512+512). K/V computed per-core for the full
sequence. No collectives.

v2 math: attn = 0.5 + 0.5*tanh(8*qkt), so
  wv = 0.5*(colsum(V) + tanh(8*qkt) @ V)
The tanh term is computed with fp8(e4m3) DoubleRow matmuls (t8 @ v8),
exploiting sigmoid-attention centering: t in [-1,1] symmetric halves the
fp8 quantization error vs quantizing attn in [0,1] directly. colsum(V) =
(sum_j x_j) @ Wv is precomputed on host per batch (tiny matvec) and added
via the per-partition-scalar port of DVE/ACT. The global 0.5 is folded
into Wo on host. Output projection accumulates over heads in PSUM.

Optional VR8 two-term V split (v ~ v8 + vr8) for ~4x lower fp8 error at
the cost of 4 extra DVE subs + 8 extra DR matmuls per head.
"""

import numpy as np

B, S, F, H = 4, 1024, 256, 8
N_CORES = 8
SQ = S // 2  # query rows per core
VR8 = False  # two-term fp8 V split

_CACHE = {}


def _build_nc():
    import concourse.mybir as mybir
    import concourse.tile as tile
    from concourse import bacc
    from concourse.tile_rust import add_dep_helper

    f32 = mybir.dt.float32
    rdt = mybir.dt.float32r
    f8 = mybir.dt.float8e4
    DR = mybir.MatmulPerfMode.DoubleRow
    Tanh = mybir.ActivationFunctionType.Tanh
    Relu = mybir.ActivationFunctionType.Relu
    Ident = mybir.ActivationFunctionType.Identity

    nc = bacc.Bacc()
    xqT = nc.declare_dram_parameter("xqT", [F, SQ], rdt, isOutput=False)
    xoT = nc.declare_dram_parameter("xoT", [F, SQ], rdt, isOutput=False)
    Wq = nc.declare_dram_parameter("Wq", [H, F, F], rdt, isOutput=False)
    Wk = nc.declare_dram_parameter("Wk", [H, F, F], rdt, isOutput=False)
    Wv = nc.declare_dram_parameter("Wv", [H, F, F], rdt, isOutput=False)
    Wo = nc.declare_dram_parameter("Wo", [H, F, F], rdt, isOutput=False)
    csum_d = nc.declare_dram_parameter("csum", [128, 2 * H], f32, isOutput=False)
    out_d = nc.declare_dram_parameter("out", [SQ, F], f32, isOutput=True)

    with tile.TileContext(nc) as tc:
        with (
            tc.tile_pool(name="const", bufs=1) as const,
            tc.tile_pool(name="sb", bufs=2) as sb,
            tc.tile_pool(name="osb", bufs=1) as osb,
            tc.tile_pool(name="psB", bufs=3, space="PSUM") as psB,
            tc.tile_pool(name="psP", bufs=2, space="PSUM") as psP,
        ):
            # preload the activation table during the initial DMA wait
            atl0 = const.tile([128, 1], f32, name="atl0", tag="atl0")
            atl1 = const.tile([128, 1], f32, name="atl1", tag="atl1")
            nc.vector.memset(atl0[:], 0.0)
            nc.scalar.activation(atl1[:], atl0[:], Tanh, scale=1.0)

            # persistent activations: features on partitions, 2 chunks of 128.
            # head-0 prologue DMAs in consumption order: the first matmul
            # needs only xq0+wq0, the second xq1+wq1 — interleave them.
            xq = [
                const.tile([128, SQ], rdt, name=f"xq{kk}", tag=f"xq{kk}")
                for kk in range(2)
            ]
            xo = [
                const.tile([128, SQ], rdt, name=f"xo{kk}", tag=f"xo{kk}")
                for kk in range(2)
            ]
            csum = const.tile([128, 2 * H], f32, name="csum", tag="csum")
            xhalves = [xq, xo]

            # persistent output-projection accumulators (PSUM, 2 banks):
            # PO[t][:, (mq%2)*256:...] accumulates q-chunk mq over all heads.
            # Two accumulation groups share each bank, and PSUM zero-regions
            # are bank-wide: a start=True from one group wipes the other
            # group's accumulated data at its next write. So: memset once,
            # accumulate with start=False everywhere.
            PO = []
            for t in range(2):
                po = psP.tile([128, 512], f32, name=f"PO{t}", tag="po")
                nc.vector.memset(po[:], 0.0)
                PO.append(po)

            state = {"prev_w_dma": None}

            def alloc_weights():
                return {
                    nm: [
                        sb.tile(
                            [128, F], rdt, name=f"{nm}{kk}", tag=f"{nm}{kk}", bufs=3
                        )
                        for kk in range(2)
                    ]
                    for nm in ("wq", "wk", "wv", "wo")
                }

            def dma_weights(h, ws=None):
                # weight tiles for head h; issued one head ahead.
                # Chain transfers in consumption order so HBM bandwidth
                # isn't fair-shared across queues.
                if ws is None:
                    ws = alloc_weights()
                dmas = []
                for nm, dram, eng in (
                    ("wq", Wq, nc.sync),
                    ("wk", Wk, nc.sync),
                    ("wv", Wv, nc.gpsimd),
                    ("wo", Wo, nc.gpsimd),
                ):
                    for kk in range(2):
                        d = eng.dma_start(
                            out=ws[nm][kk][:],
                            in_=dram[h, kk * 128 : (kk + 1) * 128, :],
                        )
                        dmas.append(d)
                gate = state["prev_w_dma"].ins
                for d in dmas:
                    add_dep_helper(d.ins, gate, reason="hbm priority")
                state["prev_w_dma"] = dmas[-1]
                return ws

            def prologue_dmas(ws0):
                # sync queue, consumption order: xq0, wq0, xq1, wq1, wk0, wk1
                d_xq0 = nc.sync.dma_start(out=xq[0][:], in_=xqT[0:128, :])
                d_wq0 = nc.sync.dma_start(out=ws0["wq"][0][:], in_=Wq[0, 0:128, :])
                nc.sync.dma_start(out=xq[1][:], in_=xqT[128:256, :])
                d_wq1 = nc.sync.dma_start(out=ws0["wq"][1][:], in_=Wq[0, 128:256, :])
                nc.sync.dma_start(out=ws0["wk"][0][:], in_=Wk[0, 0:128, :])
                nc.sync.dma_start(out=ws0["wk"][1][:], in_=Wk[0, 128:256, :])
                # gpsimd queue, gated behind wq1: xo, wv, wo, csum
                gdmas = []
                for kk in range(2):
                    gdmas.append(
                        nc.gpsimd.dma_start(
                            out=xo[kk][:], in_=xoT[kk * 128 : (kk + 1) * 128, :]
                        )
                    )
                for nm, dram in (("wv", Wv), ("wo", Wo)):
                    for kk in range(2):
                        gdmas.append(
                            nc.gpsimd.dma_start(
                                out=ws0[nm][kk][:],
                                in_=dram[0, kk * 128 : (kk + 1) * 128, :],
                            )
                        )
                d_csum = nc.gpsimd.dma_start(out=csum[:], in_=csum_d[:])
                gdmas.append(d_csum)
                for d in gdmas:
                    add_dep_helper(d.ins, d_wq1.ins, reason="hbm priority")
                state["prev_w_dma"] = gdmas[-2]  # wo1

            def qkv_gen(h, ws):
                """Generator emitting Q/K/V phases for head h in 5 segments."""
                wq, wk, wv = ws["wq"], ws["wk"], ws["wv"]
                r = {}
                # -- Q: psq [128, 2x512], qt [128,1024] f32r (m-major) --
                psq = psB.tile([128, 1024], f32, name="psq", tag="B")
                for m in range(2):
                    for kk in range(2):
                        nc.tensor.matmul(
                            psq[:, m * 512 : (m + 1) * 512],
                            wq[kk][:, m * 128 : (m + 1) * 128],
                            xq[kk][:],
                            start=(kk == 0),
                            stop=(kk == 1),
                        )
                qt = sb.tile([128, 1024], rdt, name="qt", tag="qt")
                nc.vector.tensor_copy(qt[:], psq[:])
                r["qt"] = qt
                yield r
                # -- K chunk m=0: psk0 [128, 1024 j] --
                kT = []
                for m in range(2):
                    psk = psB.tile([128, 1024], f32, name=f"psk{m}", tag="B")
                    for n in range(2):
                        for kk in range(2):
                            nc.tensor.matmul(
                                psk[:, n * 512 : (n + 1) * 512],
                                wk[kk][:, m * 128 : (m + 1) * 128],
                                xhalves[n][kk][:],
                                start=(kk == 0),
                                stop=(kk == 1),
                            )
                    t = sb.tile([128, 1024], rdt, name=f"kT{m}", tag=f"kT{m}")
                    if m == 0:
                        nc.scalar.copy(t[:], psk[:])
                    else:
                        nc.vector.tensor_copy(t[:], psk[:])
                    kT.append(t)
                    r["kT"] = kT
                    yield r
                # -- V: pairs p=0..3; v8p[p] [128,2,256] fp8 (+vr8) --
                v8p, vr8p = [], []
                for half in range(2):
                    VP = psB.tile([128, 1024], f32, name=f"VP{half}", tag="B")
                    for p2 in range(2):
                        p = half * 2 + p2
                        for t2 in range(2):
                            jb = p * 2 + t2
                            xh = xhalves[jb // 4]
                            jj = jb % 4
                            for kk in range(2):
                                nc.tensor.matmul(
                                    VP[:, (p2 * 2 + t2) * 256 : (p2 * 2 + t2 + 1) * 256],
                                    xh[kk][:, jj * 128 : (jj + 1) * 128],
                                    wv[kk][:],
                                    start=(kk == 0),
                                    stop=(kk == 1),
                                )
                        v8 = sb.tile([128, 2, 256], f8, name=f"v8p{p}", tag=f"v8p{p}")
                        nc.vector.tensor_copy(
                            v8[:, :, :], VP[:, p2 * 512 : (p2 + 1) * 512]
                        )
                        v8p.append(v8)
                        if VR8:
                            vr = sb.tile(
                                [128, 2, 256], f8, name=f"vr8p{p}", tag=f"vr8p{p}"
                            )
                            nc.vector.tensor_sub(
                                vr[:, :, :],
                                VP[:, p2 * 512 : (p2 + 1) * 512],
                                v8[:, :, :],
                            )
                            vr8p.append(vr)
                    r["v8p"] = v8p
                    r["vr8p"] = vr8p
                    yield r

            def scores(h, r, p):
                # scores pair p -> psum [128, 2x512] -> tanh -> t8 fp8
                kT, qt = r["kT"], r["qt"]
                SP = psB.tile([128, 1024], f32, name=f"SP{p}", tag="B")
                for t2 in range(2):
                    jb = p * 2 + t2
                    for m in range(2):
                        nc.tensor.matmul(
                            SP[:, t2 * 512 : (t2 + 1) * 512],
                            kT[m][:, jb * 128 : (jb + 1) * 128],
                            qt[:, m * 512 : (m + 1) * 512],
                            start=(m == 0),
                            stop=(m == 1),
                        )
                t8 = sb.tile([128, 2, 512], f8, name=f"t8_{p}", tag=f"t8_{p}")
                nc.scalar.activation(t8[:, :, :], SP[:], Tanh, scale=8.0)
                return t8

            def ot_contrib(r, OMB, p, t8s):
                # each m-half of OMB is its own accumulation group
                last = not VR8 and p == 3
                for m in range(2):
                    nc.tensor.matmul(
                        OMB[:, m * 512 : (m + 1) * 512],
                        r["v8p"][p][:, :, m * 128 : (m + 1) * 128],
                        t8s[p][:, :, :],
                        start=(p == 0),
                        stop=last,
                        perf_mode=DR,
                    )
                if VR8:
                    for m in range(2):
                        nc.tensor.matmul(
                            OMB[:, m * 512 : (m + 1) * 512],
                            r["vr8p"][p][:, :, m * 128 : (m + 1) * 128],
                            t8s[p][:, :, :],
                            start=False,
                            stop=(p == 3),
                            perf_mode=DR,
                        )

            def oproj(h, ot, wo):
                for mq in range(4):
                    po = PO[mq // 2][:, (mq % 2) * 256 : (mq % 2 + 1) * 256]
                    for m in range(2):
                        nc.tensor.matmul(
                            po,
                            ot[m][:, mq * 128 : (mq + 1) * 128],
                            wo[m][:],
                            start=False,
                            stop=(h == H - 1 and m == 1),
                            skip_group_check=True,
                        )
                    if h == H - 1:
                        o = osb.tile([128, F], f32, name=f"outsb{mq}", tag=f"outsb{mq}")
                        nc.scalar.activation(o[:], po, Relu)
                        nc.sync.dma_start(
                            out=out_d[mq * 128 : (mq + 1) * 128, :], in_=o[:]
                        )

            # ---------------- main pipeline ----------------
            ws_cur = alloc_weights()
            prologue_dmas(ws_cur)
            g = qkv_gen(0, ws_cur)
            r = None
            for r in g:
                pass
            for h in range(H):
                ws_nxt = dma_weights(h + 1) if h < H - 1 else None
                gn = qkv_gen(h + 1, ws_nxt) if h < H - 1 else None

                def step():
                    if gn is not None:
                        return next(gn, None)
                    return None

                t8s = [scores(h, r, p) for p in range(4)]
                OMB = psB.tile([128, 1024], f32, name="OMB", tag="B")
                ot_contrib(r, OMB, 0, t8s)
                ot_contrib(r, OMB, 1, t8s)
                rn = step()  # Q h+1
                ot_contrib(r, OMB, 2, t8s)
                rn = step() or rn  # K0 h+1
                ot_contrib(r, OMB, 3, t8s)
                rn = step() or rn  # K1 h+1

                # ot = OMB + colsum (per-partition scalar), f32r.
                # Last head: split into halves so the first Oproj matmuls
                # (which only read cols 0:128) unlock sooner via subtile deps.
                ot = []
                t0 = sb.tile([128, 512], rdt, name="ot0", tag="ot0")
                t1 = sb.tile([128, 512], rdt, name="ot1", tag="ot1")
                if h == H - 1:
                    for hf in range(2):
                        sl = slice(hf * 256, (hf + 1) * 256)
                        nc.vector.tensor_scalar_add(
                            t0[:, sl], OMB[:, 0:512][:, sl],
                            csum[:, 2 * h : 2 * h + 1],
                        )
                        nc.scalar.activation(
                            t1[:, sl], OMB[:, 512:1024][:, sl], Ident,
                            bias=csum[:, 2 * h + 1 : 2 * h + 2],
                        )
                else:
                    nc.vector.tensor_scalar_add(
                        t0[:], OMB[:, 0:512], csum[:, 2 * h : 2 * h + 1]
                    )
                    nc.scalar.activation(
                        t1[:], OMB[:, 512:1024], Ident,
                        bias=csum[:, 2 * h + 1 : 2 * h + 2],
                    )
                ot.append(t0)
                ot.append(t1)

                rn = step() or rn  # V first half h+1
                rn = step() or rn  # V second half h+1

                oproj(h, ot, ws_cur["wo"])
                if gn is not None:
                    for rr in gn:
                        rn = rr
                    r = rn
                    ws_cur = ws_nxt

    nc.finalize()
    return nc


def _get_nc():
    if "nc" not in _CACHE:
        _CACHE["nc"] = _build_nc()
    return _CACHE["nc"]


def _prep_weights(Wq, Wk, Wv, Wo):
    # [F, F*H] with column f*H+h  ->  [H, F, F] head-contiguous
    wq = np.ascontiguousarray(Wq.reshape(F, F, H).transpose(2, 0, 1))
    wk = np.ascontiguousarray(Wk.reshape(F, F, H).transpose(2, 0, 1))
    wv = np.ascontiguousarray(Wv.reshape(F, F, H).transpose(2, 0, 1))
    # [F*H, F] with row f*H+h -> [H, F, F]; fold the 0.5 centering factor
    wo = np.ascontiguousarray(Wo.reshape(F, H, F).transpose(1, 0, 2)) * 0.5
    return wq, wk, wv, wo


def kernel(q_input, Wq, Wk, Wv, Wo, _trace=False):
    from concourse.bass_utils import run_bass_kernel_spmd

    nc = _get_nc()
    wq, wk, wv, wo = _prep_weights(
        np.asarray(Wq, np.float32),
        np.asarray(Wk, np.float32),
        np.asarray(Wv, np.float32),
        np.asarray(Wo, np.float32),
    )
    q_input = np.asarray(q_input, np.float32)

    in_maps = []
    for c in range(N_CORES):
        b, half = c // 2, c % 2
        xT = q_input[b].T
        xqT = np.ascontiguousarray(xT[:, half * SQ : (half + 1) * SQ])
        xoT = np.ascontiguousarray(xT[:, (1 - half) * SQ : (2 - half) * SQ])
        # colsum_h = (sum_j x[j,:]) @ Wv_h ; layout [128, h*2+m]
        xsum = q_input[b].sum(axis=0)  # [F]
        cs = (xsum @ wv.reshape(H * F, F).reshape(H, F, F)).reshape(H, 2, 128)
        csum = np.ascontiguousarray(cs.transpose(2, 0, 1).reshape(128, 2 * H))
        in_maps.append(
            {
                "xqT": xqT,
                "xoT": xoT,
                "Wq": wq,
                "Wk": wk,
                "Wv": wv,
                "Wo": wo,
                "csum": csum,
            }
        )

    res = run_bass_kernel_spmd(nc, in_maps, list(range(N_CORES)), trace=_trace)

    out = np.empty((B, S, F), np.float32)
    for c in range(N_CORES):
        b, half = c // 2, c % 2
        out[b, half * SQ : (half + 1) * SQ, :] = res.results[c]["out"]
    if _trace:
        return out, res
    return out



# revision 5
# speedup vs baseline: 1.1022x; 1.1022x over previous
"""Sigmoid self-attention Bass kernel for Trainium2, SPMD on 8 cores. v3.

Problem: B=4, S=1024, F=256, H=8
  q = (X @ Wq).reshape(b,s,f,h); k,v likewise (self-attention)
  attn = sigmoid(sqrt(F) * q.kT) per (b,h);  wv = attn @ v
  out = relu(wv_flat @ Wo)

Sharding: data-parallel over (batch, seq-half): core c handles batch c//2,
query rows [half*512, half*512+512). V computed per-core for the full
sequence. No collectives.

v3 math: scores = q k^T = x (Wq Wk^T) x^T, with M_h = Wq_h Wk_h^T folded
on host. This removes the K projection and kT materialization entirely:
per head only qm = M_h^T x_q^T is computed ([256, 512]), and the scores
matmul contracts x^T (already resident for V) against qm.

attn = 0.5 + 0.5*tanh(8*qkt), so wv = 0.5*(colsum(V) + tanh(8*qkt) @ V).
The tanh term uses fp8(e4m3) DoubleRow matmuls (t8 @ v8). colsum(V) =
(sum_j x_j) @ Wv is precomputed on host per batch and added via the
per-partition-scalar port of DVE/ACT. The 0.5 is folded into Wo on host.
Output projection accumulates over heads in PSUM.

Tail: the first-processed head's (actual head 7) oproj is deferred to the
end, so the final chain after the last head's attention is only
oproj(h6)+oproj(h7) -> relu -> DMA, pipelined per 128-row q-chunk over
Scalar/Vector relu and 4 DMA queues.
"""

import numpy as np

B, S, F, H = 4, 1024, 256, 8
N_CORES = 8
SQ = S // 2  # query rows per core

_CACHE = {}


def _build_nc():
    import concourse.mybir as mybir
    import concourse.tile as tile
    from concourse import bacc
    from concourse.tile_rust import add_dep_helper

    f32 = mybir.dt.float32
    rdt = mybir.dt.float32r
    f8 = mybir.dt.float8e4
    DR = mybir.MatmulPerfMode.DoubleRow
    Tanh = mybir.ActivationFunctionType.Tanh
    Relu = mybir.ActivationFunctionType.Relu
    Ident = mybir.ActivationFunctionType.Identity

    PH = [7, 0, 1, 2, 3, 4, 5, 6]  # program head order; h=7 oproj deferred

    nc = bacc.Bacc()
    xqT = nc.declare_dram_parameter("xqT", [128, 2, SQ], rdt, isOutput=False)
    xoT = nc.declare_dram_parameter("xoT", [128, 2, SQ], rdt, isOutput=False)
    Wm = nc.declare_dram_parameter("Wm", [H, 128, 2, F], rdt, isOutput=False)
    Wv = nc.declare_dram_parameter("Wv", [H, 128, 2, F], rdt, isOutput=False)
    Wo = nc.declare_dram_parameter("Wo", [H, 128, 2, F], rdt, isOutput=False)
    csum_d = nc.declare_dram_parameter("csum", [128, 2 * H], f32, isOutput=False)
    out_d = nc.declare_dram_parameter("out", [SQ, F], f32, isOutput=True)

    with tile.TileContext(nc) as tc:
        with (
            tc.tile_pool(name="const", bufs=1) as const,
            tc.tile_pool(name="sb", bufs=2) as sb,
            tc.tile_pool(name="osb", bufs=1) as osb,
            tc.tile_pool(name="psB", bufs=3, space="PSUM") as psB,
            tc.tile_pool(name="psP", bufs=2, space="PSUM") as psP,
        ):
            # persistent activations: features on partitions, [128, kk, s]
            xq = const.tile([128, 2, SQ], rdt, name="xq", tag="xq")
            xo = const.tile([128, 2, SQ], rdt, name="xo", tag="xo")
            csum = const.tile([128, 2 * H], f32, name="csum", tag="csum")
            # head-7 tiles that must survive until the tail
            wo7 = const.tile([128, 2, F], rdt, name="wo7", tag="wo7")
            ot7 = [
                const.tile([128, SQ], rdt, name=f"ot7_{m}", tag=f"ot7_{m}")
                for m in range(2)
            ]

            # persistent output-projection accumulators (PSUM, 2 banks):
            # PO[t][:, (mq%2)*256:...] accumulates q-chunk mq over all heads.
            # PSUM zero-regions are bank-wide: memset once, start=False
            # everywhere.
            PO = []
            for t in range(2):
                po = psP.tile([128, 512], f32, name=f"PO{t}", tag="po")
                nc.vector.memset(po[:], 0.0)
                PO.append(po)

            state = {"prev_w_dma": None}

            def alloc_weights(h):
                return {
                    nm: sb.tile([128, 2, F], rdt, name=f"{nm}{h}", tag=nm, bufs=3)
                    for nm in ("wm", "wv", "wo")
                }

            def dma_weights(h):
                # weight tiles for head h; issued two program-heads ahead.
                # Chain transfers behind the previous head's last weight DMA
                # so HBM bandwidth isn't fair-shared across queues.
                ws = alloc_weights(h)
                dmas = []
                for nm, dram, eng in (
                    ("wm", Wm, nc.sync),
                    ("wv", Wv, nc.sync),
                    ("wo", Wo, nc.gpsimd),
                ):
                    d = eng.dma_start(out=ws[nm][:, :, :], in_=dram[h])
                    dmas.append(d)
                gate = state["prev_w_dma"].ins
                for d in dmas:
                    add_dep_helper(d.ins, gate, reason="hbm priority")
                state["prev_w_dma"] = dmas[-1]
                return ws

            def prologue_dmas(ws7, ws0):
                # consumption order: wm7 (first MM) and xq first.
                # sync queue: wm7, xq halves, wm0
                nc.sync.dma_start(out=ws7["wm"][:, :, :], in_=Wm[7])
                for kk in range(2):
                    nc.sync.dma_start(out=xq[:, kk, :], in_=xqT[:, kk, :])
                nc.sync.dma_start(out=ws0["wm"][:, :, :], in_=Wm[0])
                # scalar queue: xo halves, wv0 (then the tanh table preload)
                for kk in range(2):
                    nc.scalar.dma_start(out=xo[:, kk, :], in_=xoT[:, kk, :])
                d_wv0 = nc.scalar.dma_start(out=ws0["wv"][:, :, :], in_=Wv[0])
                # gpsimd queue: wv7, csum, wo7 (tail-only), wo0
                nc.gpsimd.dma_start(out=ws7["wv"][:, :, :], in_=Wv[7])
                nc.gpsimd.dma_start(out=csum[:], in_=csum_d[:])
                nc.gpsimd.dma_start(out=wo7[:, :, :], in_=Wo[7])
                d_wo0 = nc.gpsimd.dma_start(out=ws0["wo"][:, :, :], in_=Wo[0])
                add_dep_helper(d_wo0.ins, d_wv0.ins, reason="hbm priority")
                state["prev_w_dma"] = d_wo0
                # preload the tanh activation table while DMAs are in flight
                atl0 = const.tile([128, 1], f32, name="atl0", tag="atl0")
                atl1 = const.tile([128, 1], f32, name="atl1", tag="atl1")
                nc.vector.memset(atl0[:], 0.0)
                nc.scalar.activation(atl1[:], atl0[:], Tanh, scale=1.0)

            def qm_phase(h, ws):
                # qm = M_h^T x_q^T : psq [128 (jout m), 2x512] -> qt f32r
                psq = psB.tile([128, 1024], f32, name=f"psq{h}", tag="B")
                for kk in range(2):
                    for m in range(2):
                        nc.tensor.matmul(
                            psq[:, m * 512 : (m + 1) * 512],
                            ws["wm"][:, kk, m * 128 : (m + 1) * 128],
                            xq[:, kk, :],
                            start=(kk == 0),
                            stop=(kk == 1),
                        )
                qt = sb.tile([128, 2, SQ], rdt, name=f"qt{h}", tag="qt")
                nc.vector.tensor_copy(qt[:, :, :], psq[:])
                return qt

            def v_phase(h, ws):
                # V for the full sequence: v8p[p] [128 j, 2, 256] fp8
                v8p = []
                for half in range(2):
                    VP = psB.tile([128, 1024], f32, name=f"VP{half}_{h}", tag="B")
                    for p2 in range(2):
                        p = half * 2 + p2
                        for t2 in range(2):
                            jb = p * 2 + t2
                            xh = xq if jb < 4 else xo
                            jj = jb % 4
                            for kk in range(2):
                                nc.tensor.matmul(
                                    VP[:, (p2 * 2 + t2) * 256 : (p2 * 2 + t2 + 1) * 256],
                                    xh[:, kk, jj * 128 : (jj + 1) * 128],
                                    ws["wv"][:, kk, :],
                                    start=(kk == 0),
                                    stop=(kk == 1),
                                )
                        v8 = sb.tile([128, 2, 256], f8, name=f"v8p{p}_{h}", tag=f"v8p{p}")
                        nc.vector.tensor_copy(
                            v8[:, :, :], VP[:, p2 * 512 : (p2 + 1) * 512]
                        )
                        v8p.append(v8)
                return v8p

            def scores(h, qt, p):
                # scores pair p -> psum [128 j, 2x512 q] -> tanh -> t8 fp8
                SP = psB.tile([128, 1024], f32, name=f"SP{p}_{h}", tag="B")
                for t2 in range(2):
                    jb = p * 2 + t2
                    xh = xq if jb < 4 else xo
                    jj = jb % 4
                    for m in range(2):
                        nc.tensor.matmul(
                            SP[:, t2 * 512 : (t2 + 1) * 512],
                            xh[:, m, jj * 128 : (jj + 1) * 128],
                            qt[:, m, :],
                            start=(m == 0),
                            stop=(m == 1),
                        )
                t8 = sb.tile([128, 2, 512], f8, name=f"t8_{p}_{h}", tag=f"t8_{p}")
                nc.scalar.activation(t8[:, :, :], SP[:], Tanh, scale=8.0)
                return t8

            def ot_contrib(v8p, OMB, p, t8s):
                # each m-half of OMB is its own accumulation group
                for m in range(2):
                    nc.tensor.matmul(
                        OMB[:, m * 512 : (m + 1) * 512],
                        v8p[p][:, :, m * 128 : (m + 1) * 128],
                        t8s[p][:, :, :],
                        start=(p == 0),
                        stop=(p == 3),
                        perf_mode=DR,
                    )

            def csum_add(h, OMB, dest, split):
                # ot = OMB + colsum (per-partition scalar), f32r.
                # split: number of column chunks per m-half (for subtile
                # unlocking of the tail oproj).
                w = 512 // split
                for c in range(split):
                    sl = slice(c * w, (c + 1) * w)
                    nc.vector.tensor_scalar_add(
                        dest[0][:, sl], OMB[:, 0:512][:, sl],
                        csum[:, 2 * h : 2 * h + 1],
                    )
                    nc.scalar.activation(
                        dest[1][:, sl], OMB[:, 512:1024][:, sl], Ident,
                        bias=csum[:, 2 * h + 1 : 2 * h + 2],
                    )

            def oproj(h, ot, wo, stop=False):
                for mq in range(4):
                    po = PO[mq // 2][:, (mq % 2) * 256 : (mq % 2 + 1) * 256]
                    for m in range(2):
                        nc.tensor.matmul(
                            po,
                            ot[m][:, mq * 128 : (mq + 1) * 128],
                            wo[:, m, :],
                            start=False,
                            stop=(stop and m == 1),
                            skip_group_check=True,
                        )

            def tail_epilogue(h6, ot6, wo6):
                # fused oproj(h6)+oproj(h7, stop) -> relu -> DMA, per q-chunk
                dma_eng = [nc.sync, nc.gpsimd, nc.scalar, nc.sync]
                for mq in range(4):
                    po = PO[mq // 2][:, (mq % 2) * 256 : (mq % 2 + 1) * 256]
                    for m in range(2):
                        nc.tensor.matmul(
                            po,
                            ot6[m][:, mq * 128 : (mq + 1) * 128],
                            wo6[:, m, :],
                            start=False,
                            stop=False,
                            skip_group_check=True,
                        )
                    for m in range(2):
                        nc.tensor.matmul(
                            po,
                            ot7[m][:, mq * 128 : (mq + 1) * 128],
                            wo7[:, m, :],
                            start=False,
                            stop=(m == 1),
                            skip_group_check=True,
                        )
                    o = osb.tile([128, F], f32, name=f"outsb{mq}", tag=f"outsb{mq}")
                    if mq % 2 == 0:
                        nc.scalar.activation(o[:], po, Relu)
                    else:
                        nc.vector.tensor_scalar_max(o[:], po, 0.0)
                    dma_eng[mq].dma_start(
                        out=out_d[mq * 128 : (mq + 1) * 128, :], in_=o[:]
                    )

            # ---------------- main pipeline ----------------
            ws = {7: alloc_weights(7), 0: alloc_weights(0)}
            prologue_dmas(ws[7], ws[0])
            qt_cur = qm_phase(7, ws[7])
            v8_cur = v_phase(7, ws[7])

            ot_prev = None  # (h, ot tiles, wo tile) pending oproj
            for i in range(8):
                h = PH[i]
                h_nxt = PH[i + 1] if i < 7 else None
                h_pre = PH[i + 2] if i < 6 else None
                if h_pre is not None:
                    ws[h_pre] = dma_weights(h_pre)

                t8s = [scores(h, qt_cur, p) for p in range(4)]
                OMB = psB.tile([128, 1024], f32, name=f"OMB{h}", tag="B")
                ot_contrib(v8_cur, OMB, 0, t8s)
                ot_contrib(v8_cur, OMB, 1, t8s)
                if h_nxt is not None:
                    qt_nxt = qm_phase(h_nxt, ws[h_nxt])
                ot_contrib(v8_cur, OMB, 2, t8s)
                ot_contrib(v8_cur, OMB, 3, t8s)

                # ot = OMB + colsum
                if i == 0:
                    ot = ot7
                    csum_add(h, OMB, ot, split=1)
                elif i == 7:
                    ot = [
                        sb.tile([128, SQ], rdt, name=f"ot{m}_{h}", tag=f"ot{m}")
                        for m in range(2)
                    ]
                    csum_add(h, OMB, ot, split=4)
                else:
                    ot = [
                        sb.tile([128, SQ], rdt, name=f"ot{m}_{h}", tag=f"ot{m}")
                        for m in range(2)
                    ]
                    csum_add(h, OMB, ot, split=1)

                if h_nxt is not None:
                    v8_nxt = v_phase(h_nxt, ws[h_nxt])

                if i == 7:
                    tail_epilogue(h, ot, ws[h]["wo"])
                elif i > 0:
                    oproj(h, ot, ws[h]["wo"])

                if h_nxt is not None:
                    qt_cur, v8_cur = qt_nxt, v8_nxt

    nc.finalize()
    return nc


def _get_nc():
    if "nc" not in _CACHE:
        _CACHE["nc"] = _build_nc()
    return _CACHE["nc"]


def _prep_weights(Wq, Wk, Wv, Wo):
    # [F, F*H] with column f_out*H+h  ->  per-head [f_in, f_out]
    wqh = Wq.reshape(F, F, H).transpose(2, 0, 1)  # [H, f_in, f_out]
    wkh = Wk.reshape(F, F, H).transpose(2, 0, 1)
    wvh = np.ascontiguousarray(Wv.reshape(F, F, H).transpose(2, 0, 1))
    # M_h = Wq_h @ Wk_h^T : [H, f_in_q, f_in_k]
    M = np.matmul(np.ascontiguousarray(wqh), wkh.transpose(0, 2, 1))
    # [H, 256, 256] -> [H, 128, 2, 256] (partition-major chunk interleave)
    def lay(w):
        return np.ascontiguousarray(
            w.reshape(H, 2, 128, F).transpose(0, 2, 1, 3)
        )
    # [F*H, F] with row f*H+h -> [H, F, F]; fold the 0.5 centering factor
    woh = Wo.reshape(F, H, F).transpose(1, 0, 2) * 0.5
    return lay(M), lay(wvh), lay(woh), wvh


def kernel(q_input, Wq, Wk, Wv, Wo, _trace=False):
    from concourse.bass_utils import run_bass_kernel_spmd

    nc = _get_nc()
    wm, wv, wo, wvh = _prep_weights(
        np.asarray(Wq, np.float32),
        np.asarray(Wk, np.float32),
        np.asarray(Wv, np.float32),
        np.asarray(Wo, np.float32),
    )
    q_input = np.asarray(q_input, np.float32)

    in_maps = []
    for c in range(N_CORES):
        b, half = c // 2, c % 2
        xT = q_input[b].T  # [F, S]

        def lay_x(cols):
            return np.ascontiguousarray(
                cols.reshape(2, 128, SQ).transpose(1, 0, 2)
            )
        xqT = lay_x(xT[:, half * SQ : (half + 1) * SQ])
        xoT = lay_x(xT[:, (1 - half) * SQ : (2 - half) * SQ])
        # colsum_h = (sum_j x[j,:]) @ Wv_h ; layout [128, h*2+m]
        xsum = q_input[b].sum(axis=0)  # [F]
        cs = (xsum @ wvh).reshape(H, 2, 128)
        csum = np.ascontiguousarray(cs.transpose(2, 0, 1).reshape(128, 2 * H))
        in_maps.append(
            {
                "xqT": xqT,
                "xoT": xoT,
                "Wm": wm,
                "Wv": wv,
                "Wo": wo,
                "csum": csum,
            }
        )

    res = run_bass_kernel_spmd(nc, in_maps, list(range(N_CORES)), trace=_trace)

    out = np.empty((B, S, F), np.float32)
    for c in range(N_CORES):
        b, half = c // 2, c % 2
        out[b, half * SQ : (half + 1) * SQ, :] = res.results[c]["out"]
    if _trace:
        return out, res
    return out


# revision 12
# speedup vs baseline: 1.1073x; 1.0046x over previous
"""Sigmoid self-attention Bass kernel for Trainium2, SPMD on 8 cores. v4.

Problem: B=4, S=1024, F=256, H=8
  q = (X @ Wq).reshape(b,s,f,h); k,v likewise (self-attention)
  attn = sigmoid(sqrt(F) * q.kT) per (b,h);  wv = attn @ v
  out = relu(wv_flat @ Wo)

Sharding: data-parallel over (batch, seq-half): core c handles batch c//2,
query rows [half*512, half*512+512). V computed per-core for the full
sequence. No collectives.

v4 math: scores = q k^T = x (Wq Wk^T) x^T, with M_h = Wq_h Wk_h^T folded
on host. This removes the K projection and kT materialization entirely:
per head only qm = M_h^T x_q^T is computed ([256, 512]), and the scores
matmul contracts x^T (already resident for V) against qm.

attn = 0.5 + 0.5*tanh(8*qkt), so wv = 0.5*(colsum(V) + tanh(8*qkt) @ V).
The tanh term uses fp8(e4m3) DoubleRow matmuls (t8 @ v8). colsum(V) =
(sum_j x_j) @ Wv is precomputed on host per batch and added via the
per-partition-scalar port of DVE/ACT. The 0.5 is folded into Wo on host.
Output projection accumulates over heads in PSUM.

Schedule: head h's block also emits qm/V for head h+1 so the PE never
waits on the DVE qt copy or the tanh chain. Prologue DMAs are chained in
consumption order across queues (HBM bw is shared; fair-sharing would
starve the first matmul). Tail (head 7): tanh/attnV of the last score
pair split by q-halves, csum adds split by q-chunk, then per-chunk
oproj -> relu (Scalar/Vector alternating) -> DMA (3 queues).
"""

import numpy as np

B, S, F, H = 4, 1024, 256, 8
N_CORES = 8
SQ = S // 2  # query rows per core

_CACHE = {}


def _build_nc():
    import concourse.mybir as mybir
    import concourse.tile as tile
    from concourse import bacc
    from concourse.tile_rust import add_dep_helper

    f32 = mybir.dt.float32
    rdt = mybir.dt.float32r
    f8 = mybir.dt.float8e4
    DR = mybir.MatmulPerfMode.DoubleRow
    Tanh = mybir.ActivationFunctionType.Tanh
    Relu = mybir.ActivationFunctionType.Relu
    Ident = mybir.ActivationFunctionType.Identity

    nc = bacc.Bacc()
    xqT = nc.declare_dram_parameter("xqT", [128, 2, SQ], rdt, isOutput=False)
    xoT = nc.declare_dram_parameter("xoT", [128, 2, SQ], rdt, isOutput=False)
    Wm = nc.declare_dram_parameter("Wm", [H, 128, 2, F], rdt, isOutput=False)
    Wv = nc.declare_dram_parameter("Wv", [H, 128, 2, F], rdt, isOutput=False)
    Wo = nc.declare_dram_parameter("Wo", [H, 128, 2, F], rdt, isOutput=False)
    csum_d = nc.declare_dram_parameter("csum", [128, 2 * H], f32, isOutput=False)
    out_d = nc.declare_dram_parameter("out", [SQ, F], f32, isOutput=True)

    with tile.TileContext(nc) as tc:
        with (
            tc.tile_pool(name="const", bufs=1) as const,
            tc.tile_pool(name="sb", bufs=2) as sb,
            tc.tile_pool(name="osb", bufs=1) as osb,
            tc.tile_pool(name="psB", bufs=3, space="PSUM") as psB,
            tc.tile_pool(name="psP", bufs=2, space="PSUM") as psP,
        ):
            # persistent activations: features on partitions, [128, kk, s]
            xq = const.tile([128, 2, SQ], rdt, name="xq", tag="xq")
            xo = const.tile([128, 2, SQ], rdt, name="xo", tag="xo")
            csum = const.tile([128, 2 * H], f32, name="csum", tag="csum")

            # persistent output-projection accumulators (PSUM, 2 banks):
            # PO[t][:, (mq%2)*256:...] accumulates q-chunk mq over all heads.
            # PSUM zero-regions are bank-wide: memset once, start=False
            # everywhere.
            PO = []
            po_msets = []
            for t in range(2):
                po = psP.tile([128, 512], f32, name=f"PO{t}", tag="po")
                po_msets.append(nc.vector.memset(po[:], 0.0))
                PO.append(po)

            state = {"prev_w_dma": None}

            def alloc_weights(h):
                return {
                    nm: sb.tile([128, 2, F], rdt, name=f"{nm}{h}", tag=nm, bufs=3)
                    for nm in ("wm", "wv", "wo")
                }

            def dma_weights(h):
                # weight tiles for head h; issued two heads ahead. Chain
                # transfers behind the previous head's last weight DMA so HBM
                # bandwidth isn't fair-shared across queues.
                ws = alloc_weights(h)
                dmas = []
                for nm, dram, eng in (
                    ("wm", Wm, nc.sync),
                    ("wv", Wv, nc.sync),
                    ("wo", Wo, nc.gpsimd),
                ):
                    d = eng.dma_start(out=ws[nm][:, :, :], in_=dram[h])
                    dmas.append(d)
                gate = state["prev_w_dma"].ins
                for d in dmas:
                    add_dep_helper(d.ins, gate, reason="hbm priority")
                state["prev_w_dma"] = dmas[-1]
                return ws

            def prologue_dmas(ws0, ws1):
                # HBM bandwidth is shared across queues: serialize transfers
                # in consumption order (cross-queue gating via add_dep_helper)
                # so the first matmul's deps arrive first.
                # sync queue, ungated: first-matmul deps then wv0.
                nc.sync.dma_start(out=ws0["wm"][:, 0, :], in_=Wm[0][:, 0, :])
                nc.sync.dma_start(out=xq[:, 0, :], in_=xqT[:, 0, :])
                nc.sync.dma_start(out=ws0["wm"][:, 1, :], in_=Wm[0][:, 1, :])
                nc.sync.dma_start(out=xq[:, 1, :], in_=xqT[:, 1, :])
                d_wv0 = nc.sync.dma_start(out=ws0["wv"][:, :, :], in_=Wv[0])
                # scalar queue, gated behind wv0: xo halves (V0 second half /
                # SP0 p2-3), then wm1.
                d_xo = []
                for kk in range(2):
                    d_xo.append(
                        nc.scalar.dma_start(out=xo[:, kk, :], in_=xoT[:, kk, :])
                    )
                d_wm1 = nc.scalar.dma_start(out=ws1["wm"][:, :, :], in_=Wm[1])
                for d in (*d_xo, d_wm1):
                    add_dep_helper(d.ins, d_wv0.ins, reason="hbm priority")
                # gpsimd queue, gated behind xo1: csum, wv1, wo0, wo1.
                g = []
                g.append(nc.gpsimd.dma_start(out=csum[:], in_=csum_d[:]))
                g.append(nc.gpsimd.dma_start(out=ws1["wv"][:, :, :], in_=Wv[1]))
                g.append(nc.gpsimd.dma_start(out=ws0["wo"][:, :, :], in_=Wo[0]))
                g.append(nc.gpsimd.dma_start(out=ws1["wo"][:, :, :], in_=Wo[1]))
                for d in g:
                    add_dep_helper(d.ins, d_xo[1].ins, reason="hbm priority")
                state["prev_w_dma"] = g[-1]  # wo1
                # preload the tanh activation table while DMAs are in flight
                atl0 = const.tile([128, 1], f32, name="atl0", tag="atl0")
                atl1 = const.tile([128, 1], f32, name="atl1", tag="atl1")
                nc.vector.memset(atl0[:], 0.0)
                nc.scalar.activation(atl1[:], atl0[:], Tanh, scale=1.0)

            def qm_phase(h, ws):
                # qm = M_h^T x_q^T : psq [128 (jout m), 2x512] -> qt f32r
                psq = psB.tile([128, 1024], f32, name=f"psq{h}", tag="B")
                for kk in range(2):
                    for m in range(2):
                        nc.tensor.matmul(
                            psq[:, m * 512 : (m + 1) * 512],
                            ws["wm"][:, kk, m * 128 : (m + 1) * 128],
                            xq[:, kk, :],
                            start=(kk == 0),
                            stop=(kk == 1),
                        )
                qt = sb.tile([128, 2, SQ], rdt, name=f"qt{h}", tag="qt")
                nc.vector.tensor_copy(qt[:, :, :], psq[:])
                return qt

            def v_phase(h, ws):
                # V for the full sequence: v8p[p] [128 j, 2, 256] fp8
                v8p = []
                for half in range(2):
                    VP = psB.tile([128, 1024], f32, name=f"VP{half}_{h}", tag="B")
                    for p2 in range(2):
                        p = half * 2 + p2
                        for t2 in range(2):
                            jb = p * 2 + t2
                            xh = xq if jb < 4 else xo
                            jj = jb % 4
                            for kk in range(2):
                                nc.tensor.matmul(
                                    VP[:, (p2 * 2 + t2) * 256 : (p2 * 2 + t2 + 1) * 256],
                                    xh[:, kk, jj * 128 : (jj + 1) * 128],
                                    ws["wv"][:, kk, :],
                                    start=(kk == 0),
                                    stop=(kk == 1),
                                )
                        v8 = sb.tile([128, 2, 256], f8, name=f"v8p{p}_{h}", tag=f"v8p{p}")
                        nc.vector.tensor_copy(
                            v8[:, :, :], VP[:, p2 * 512 : (p2 + 1) * 512]
                        )
                        v8p.append(v8)
                return v8p

            def scores(h, qt, p):
                # scores pair p -> psum [128 j(t2), 2, 512 q] -> tanh -> t8 fp8
                SP = psB.tile([128, 2, 512], f32, name=f"SP{p}_{h}", tag="B")
                for t2 in range(2):
                    jb = p * 2 + t2
                    xh = xq if jb < 4 else xo
                    jj = jb % 4
                    for m in range(2):
                        nc.tensor.matmul(
                            SP[:, t2, :],
                            xh[:, m, jj * 128 : (jj + 1) * 128],
                            qt[:, m, :],
                            start=(m == 0),
                            stop=(m == 1),
                        )
                t8 = sb.tile([128, 2, 512], f8, name=f"t8_{p}_{h}", tag=f"t8_{p}")
                nc.scalar.activation(t8[:, :, :], SP[:, :, :], Tanh, scale=8.0)
                return t8

            def ot_contrib(v8p, OMB, p, t8s):
                # each m-half of OMB is its own accumulation group
                for m in range(2):
                    nc.tensor.matmul(
                        OMB[:, m * 512 : (m + 1) * 512],
                        v8p[p][:, :, m * 128 : (m + 1) * 128],
                        t8s[p][:, :, :],
                        start=(p == 0),
                        stop=(p == 3),
                        perf_mode=DR,
                    )

            def csum_add(h, OMB, dest, split):
                # ot = OMB + colsum (per-partition scalar), f32r.
                # split: column chunks per m-half (subtile unlock of oproj).
                w = 512 // split
                for c in range(split):
                    sl = slice(c * w, (c + 1) * w)
                    nc.vector.tensor_scalar_add(
                        dest[0][:, sl], OMB[:, 0:512][:, sl],
                        csum[:, 2 * h : 2 * h + 1],
                    )
                    nc.scalar.activation(
                        dest[1][:, sl], OMB[:, 512:1024][:, sl], Ident,
                        bias=csum[:, 2 * h + 1 : 2 * h + 2],
                    )

            def oproj(h, ot, wo):
                for mq in range(4):
                    po = PO[mq // 2][:, (mq % 2) * 256 : (mq % 2 + 1) * 256]
                    for m in range(2):
                        mm = nc.tensor.matmul(
                            po,
                            ot[m][:, mq * 128 : (mq + 1) * 128],
                            wo[:, m, :],
                            start=False,
                            stop=False,
                            skip_group_check=True,
                        )
                        if h == 0:
                            add_dep_helper(
                                mm.ins, po_msets[mq // 2].ins,
                                reason="po zeroed before accumulation",
                            )

            def tail_epilogue(ot7, wo):
                # oproj(h7, stop) -> relu -> DMA, pipelined per q-chunk
                dma_eng = [nc.sync, nc.gpsimd, nc.scalar, nc.sync]
                for mq in range(4):
                    po = PO[mq // 2][:, (mq % 2) * 256 : (mq % 2 + 1) * 256]
                    for m in range(2):
                        nc.tensor.matmul(
                            po,
                            ot7[m][:, mq * 128 : (mq + 1) * 128],
                            wo[:, m, :],
                            start=False,
                            stop=(m == 1),
                            skip_group_check=True,
                        )
                    o = osb.tile([128, F], f32, name=f"outsb{mq}", tag=f"outsb{mq}")
                    if mq % 2 == 0:
                        nc.scalar.activation(o[:], po, Relu)
                    else:
                        nc.vector.tensor_scalar_max(o[:], po, 0.0)
                    dma_eng[mq].dma_start(
                        out=out_d[mq * 128 : (mq + 1) * 128, :], in_=o[:]
                    )

            # ---------------- main pipeline ----------------
            ws = [alloc_weights(0), alloc_weights(1)] + [None] * (H - 2)
            prologue_dmas(ws[0], ws[1])
            qt_cur = qm_phase(0, ws[0])
            v8_cur = v_phase(0, ws[0])

            for h in range(H):
                last = h == H - 1
                if h + 2 < H:
                    ws[h + 2] = dma_weights(h + 2)

                t8s = [scores(h, qt_cur, p) for p in range(4)]
                OMB = psB.tile([128, 1024], f32, name=f"OMB{h}", tag="B")
                ot_contrib(v8_cur, OMB, 0, t8s)
                ot_contrib(v8_cur, OMB, 1, t8s)
                if not last:
                    qt_nxt = qm_phase(h + 1, ws[h + 1])
                ot_contrib(v8_cur, OMB, 2, t8s)
                ot_contrib(v8_cur, OMB, 3, t8s)

                ot = [
                    sb.tile([128, SQ], rdt, name=f"ot{m}_{h}", tag=f"ot{m}")
                    for m in range(2)
                ]
                csum_add(h, OMB, ot, split=(4 if last else 1))

                if not last:
                    v8_nxt = v_phase(h + 1, ws[h + 1])
                    oproj(h, ot, ws[h]["wo"])
                    qt_cur, v8_cur = qt_nxt, v8_nxt
                else:
                    tail_epilogue(ot, ws[h]["wo"])

    nc.finalize()
    return nc


def _get_nc():
    if "nc" not in _CACHE:
        _CACHE["nc"] = _build_nc()
    return _CACHE["nc"]


def _prep_weights(Wq, Wk, Wv, Wo):
    # [F, F*H] with column f_out*H+h  ->  per-head [f_in, f_out]
    wqh = Wq.reshape(F, F, H).transpose(2, 0, 1)  # [H, f_in, f_out]
    wkh = Wk.reshape(F, F, H).transpose(2, 0, 1)
    wvh = np.ascontiguousarray(Wv.reshape(F, F, H).transpose(2, 0, 1))
    # M_h = Wq_h @ Wk_h^T : [H, f_in_q, f_in_k]
    M = np.matmul(np.ascontiguousarray(wqh), wkh.transpose(0, 2, 1))
    # [H, 256, 256] -> [H, 128, 2, 256] (partition-major chunk interleave)
    def lay(w):
        return np.ascontiguousarray(
            w.reshape(H, 2, 128, F).transpose(0, 2, 1, 3)
        )
    # [F*H, F] with row f*H+h -> [H, F, F]; fold the 0.5 centering factor
    woh = Wo.reshape(F, H, F).transpose(1, 0, 2) * 0.5
    return lay(M), lay(wvh), lay(woh), wvh


def kernel(q_input, Wq, Wk, Wv, Wo, _trace=False):
    from concourse.bass_utils import run_bass_kernel_spmd

    nc = _get_nc()
    wm, wv, wo, wvh = _prep_weights(
        np.asarray(Wq, np.float32),
        np.asarray(Wk, np.float32),
        np.asarray(Wv, np.float32),
        np.asarray(Wo, np.float32),
    )
    q_input = np.asarray(q_input, np.float32)

    in_maps = []
    for c in range(N_CORES):
        b, half = c // 2, c % 2
        xT = q_input[b].T  # [F, S]

        def lay_x(cols):
            return np.ascontiguousarray(
                cols.reshape(2, 128, SQ).transpose(1, 0, 2)
            )
        xqT = lay_x(xT[:, half * SQ : (half + 1) * SQ])
        xoT = lay_x(xT[:, (1 - half) * SQ : (2 - half) * SQ])
        # colsum_h = (sum_j x[j,:]) @ Wv_h ; layout [128, h*2+m]
        xsum = q_input[b].sum(axis=0)  # [F]
        cs = (xsum @ wvh).reshape(H, 2, 128)
        csum = np.ascontiguousarray(cs.transpose(2, 0, 1).reshape(128, 2 * H))
        in_maps.append(
            {
                "xqT": xqT,
                "xoT": xoT,
                "Wm": wm,
                "Wv": wv,
                "Wo": wo,
                "csum": csum,
            }
        )

    res = run_bass_kernel_spmd(nc, in_maps, list(range(N_CORES)), trace=_trace)

    out = np.empty((B, S, F), np.float32)
    for c in range(N_CORES):
        b, half = c // 2, c % 2
        out[b, half * SQ : (half + 1) * SQ, :] = res.results[c]["out"]
    if _trace:
        return out, res
    return out


# revision 19
# speedup vs baseline: 1.1227x; 1.0139x over previous
"""Sigmoid self-attention Bass kernel for Trainium2, SPMD on 8 cores. v4.

Problem: B=4, S=1024, F=256, H=8
  q = (X @ Wq).reshape(b,s,f,h); k,v likewise (self-attention)
  attn = sigmoid(sqrt(F) * q.kT) per (b,h);  wv = attn @ v
  out = relu(wv_flat @ Wo)

Sharding: data-parallel over (batch, seq-half): core c handles batch c//2,
query rows [half*512, half*512+512). V computed per-core for the full
sequence. No collectives.

v4 math: scores = q k^T = x (Wq Wk^T) x^T, with M_h = Wq_h Wk_h^T folded
on host. This removes the K projection and kT materialization entirely:
per head only qm = M_h^T x_q^T is computed ([256, 512]), and the scores
matmul contracts x^T (already resident for V) against qm.

attn = 0.5 + 0.5*tanh(8*qkt), so wv = 0.5*(colsum(V) + tanh(8*qkt) @ V).
The tanh term uses fp8(e4m3) DoubleRow matmuls (t8 @ v8). colsum(V) =
(sum_j x_j) @ Wv is precomputed on host per batch and added via the
per-partition-scalar port of DVE/ACT. The 0.5 is folded into Wo on host.
Output projection accumulates over heads in PSUM.

Schedule: head h's block also emits qm/V for head h+1 so the PE never
waits on the DVE qt copy or the tanh chain. Prologue DMAs are chained in
consumption order across queues (HBM bw is shared; fair-sharing would
starve the first matmul). Tail (head 7): tanh/attnV of the last score
pair split by q-halves, csum adds split by q-chunk, then per-chunk
oproj -> relu (Scalar/Vector alternating) -> DMA (3 queues).
"""

import numpy as np

B, S, F, H = 4, 1024, 256, 8
N_CORES = 8
SQ = S // 2  # query rows per core

_CACHE = {}


def _build_nc():
    import concourse.mybir as mybir
    import concourse.tile as tile
    from concourse import bacc
    from concourse.tile_rust import add_dep_helper

    f32 = mybir.dt.float32
    rdt = mybir.dt.float32r
    f8 = mybir.dt.float8e4
    DR = mybir.MatmulPerfMode.DoubleRow
    Tanh = mybir.ActivationFunctionType.Tanh
    Relu = mybir.ActivationFunctionType.Relu
    Ident = mybir.ActivationFunctionType.Identity

    nc = bacc.Bacc()
    xqT = nc.declare_dram_parameter("xqT", [128, 2, SQ], rdt, isOutput=False)
    xoT = nc.declare_dram_parameter("xoT", [128, 2, SQ], rdt, isOutput=False)
    Wm = nc.declare_dram_parameter("Wm", [H, 128, 2, F], rdt, isOutput=False)
    Wv = nc.declare_dram_parameter("Wv", [H, 128, 2, F], rdt, isOutput=False)
    Wo = nc.declare_dram_parameter("Wo", [H, 128, 2, F], rdt, isOutput=False)
    csum_d = nc.declare_dram_parameter("csum", [128, 2 * H], f32, isOutput=False)
    out_d = nc.declare_dram_parameter("out", [SQ, F], f32, isOutput=True)

    with tile.TileContext(nc) as tc:
        with (
            tc.tile_pool(name="const", bufs=1) as const,
            tc.tile_pool(name="sb", bufs=2) as sb,
            tc.tile_pool(name="osb", bufs=1) as osb,
            tc.tile_pool(name="psB", bufs=3, space="PSUM") as psB,
            tc.tile_pool(name="psP", bufs=2, space="PSUM") as psP,
        ):
            # persistent activations: features on partitions, [128, kk, s]
            xq = const.tile([128, 2, SQ], rdt, name="xq", tag="xq")
            xo = const.tile([128, 2, SQ], rdt, name="xo", tag="xo")
            csum = const.tile([128, 2 * H], f32, name="csum", tag="csum")

            # persistent output-projection accumulators (PSUM, 2 banks):
            # PO[t][:, (mq%2)*256:...] accumulates q-chunk mq over all heads.
            # PSUM zero-regions are bank-wide: memset once, start=False
            # everywhere.
            PO = []
            po_msets = []
            for t in range(2):
                po = psP.tile([128, 512], f32, name=f"PO{t}", tag="po")
                po_msets.append(nc.vector.memset(po[:], 0.0))
                PO.append(po)

            state = {"prev_w_dma": None}

            def alloc_weights(h):
                return {
                    nm: sb.tile([128, 2, F], rdt, name=f"{nm}{h}", tag=nm, bufs=3)
                    for nm in ("wm", "wv", "wo")
                }

            def dma_weights(h):
                # weight tiles for head h; issued two heads ahead. Chain
                # transfers behind the previous head's last weight DMA so HBM
                # bandwidth isn't fair-shared across queues.
                ws = alloc_weights(h)
                dmas = []
                for nm, dram, eng in (
                    ("wm", Wm, nc.sync),
                    ("wv", Wv, nc.sync),
                    ("wo", Wo, nc.gpsimd),
                ):
                    d = eng.dma_start(out=ws[nm][:, :, :], in_=dram[h])
                    dmas.append(d)
                gate = state["prev_w_dma"].ins
                for d in dmas:
                    add_dep_helper(d.ins, gate, reason="hbm priority")
                state["prev_w_dma"] = dmas[-1]
                return ws

            def prologue_dmas(ws0, ws1):
                # HBM bandwidth is shared across queues: keep all early
                # transfers on ONE queue in consumption order (per-queue
                # transfers are serial and issue pipelines with transfer),
                # so the first matmul's deps arrive first. Per-DMA latency
                # is ~2.5us fixed + transfer, so the first x chunk is
                # quarter-sized to start compute earlier.
                nc.sync.dma_start(out=ws0["wm"][:, 0, :], in_=Wm[0][:, 0, :])
                nc.sync.dma_start(out=xq[:, 0, 0:256], in_=xqT[:, 0, 0:256])
                nc.sync.dma_start(out=ws0["wm"][:, 1, :], in_=Wm[0][:, 1, :])
                nc.sync.dma_start(out=xq[:, 1, 0:256], in_=xqT[:, 1, 0:256])
                nc.sync.dma_start(out=xq[:, 0, 256:512], in_=xqT[:, 0, 256:512])
                nc.sync.dma_start(out=xq[:, 1, 256:512], in_=xqT[:, 1, 256:512])
                d_wv0 = nc.sync.dma_start(out=ws0["wv"][:, :, :], in_=Wv[0])
                d_xo = []
                for kk in range(2):
                    d_xo.append(
                        nc.sync.dma_start(out=xo[:, kk, :], in_=xoT[:, kk, :])
                    )
                # scalar queue, gated behind xo1: wm1, wv1.
                d_wm1 = nc.scalar.dma_start(out=ws1["wm"][:, :, :], in_=Wm[1])
                d_wv1 = nc.scalar.dma_start(out=ws1["wv"][:, :, :], in_=Wv[1])
                for d in (d_wm1, d_wv1):
                    add_dep_helper(d.ins, d_xo[1].ins, reason="hbm priority")
                # gpsimd queue, gated behind xo1: csum, wo0, wo1.
                g = []
                g.append(nc.gpsimd.dma_start(out=csum[:], in_=csum_d[:]))
                g.append(nc.gpsimd.dma_start(out=ws0["wo"][:, :, :], in_=Wo[0]))
                g.append(nc.gpsimd.dma_start(out=ws1["wo"][:, :, :], in_=Wo[1]))
                for d in g:
                    add_dep_helper(d.ins, d_xo[1].ins, reason="hbm priority")
                state["prev_w_dma"] = g[-1]  # wo1
                # preload the tanh activation table while DMAs are in flight
                atl0 = const.tile([128, 1], f32, name="atl0", tag="atl0")
                atl1 = const.tile([128, 1], f32, name="atl1", tag="atl1")
                nc.vector.memset(atl0[:], 0.0)
                nc.scalar.activation(atl1[:], atl0[:], Tanh, scale=1.0)

            def qm_phase(h, ws, qsplit=1):
                # qm = M_h^T x_q^T : psq [128 (jout m), 2x512] -> qt f32r
                # qsplit>1 (head 0 only): smaller moving chunks so the first
                # matmul starts as soon as the first quarter of xq lands.
                # PSUM zero-regions are bank-wide: a start=True while another
                # group in the same bank is mid-accumulation wipes it. So each
                # q-chunk's full contraction completes before the next starts.
                psq = psB.tile([128, 1024], f32, name=f"psq{h}", tag="B")
                w = SQ // qsplit
                for c in range(qsplit):
                    sl = slice(c * w, (c + 1) * w)
                    for m in range(2):
                        for kk in range(2):
                            nc.tensor.matmul(
                                psq[:, m * 512 : (m + 1) * 512][:, sl],
                                ws["wm"][:, kk, m * 128 : (m + 1) * 128],
                                xq[:, kk, sl],
                                start=(kk == 0),
                                stop=(kk == 1),
                            )
                qt = sb.tile([128, 2, SQ], rdt, name=f"qt{h}", tag="qt")
                nc.vector.tensor_copy(qt[:, :, :], psq[:])
                return qt

            def v_phase(h, ws):
                # V for the full sequence: v8p[p] [128 j, 2, 256] fp8
                v8p = []
                for half in range(2):
                    VP = psB.tile([128, 1024], f32, name=f"VP{half}_{h}", tag="B")
                    for p2 in range(2):
                        p = half * 2 + p2
                        for t2 in range(2):
                            jb = p * 2 + t2
                            xh = xq if jb < 4 else xo
                            jj = jb % 4
                            for kk in range(2):
                                nc.tensor.matmul(
                                    VP[:, (p2 * 2 + t2) * 256 : (p2 * 2 + t2 + 1) * 256],
                                    xh[:, kk, jj * 128 : (jj + 1) * 128],
                                    ws["wv"][:, kk, :],
                                    start=(kk == 0),
                                    stop=(kk == 1),
                                )
                        v8 = sb.tile([128, 2, 256], f8, name=f"v8p{p}_{h}", tag=f"v8p{p}")
                        nc.vector.tensor_copy(
                            v8[:, :, :], VP[:, p2 * 512 : (p2 + 1) * 512]
                        )
                        v8p.append(v8)
                return v8p

            def scores(h, qt, p):
                # scores pair p -> psum [128 j(t2), 2, 512 q] -> tanh -> t8 fp8
                SP = psB.tile([128, 2, 512], f32, name=f"SP{p}_{h}", tag="B")
                for t2 in range(2):
                    jb = p * 2 + t2
                    xh = xq if jb < 4 else xo
                    jj = jb % 4
                    for m in range(2):
                        nc.tensor.matmul(
                            SP[:, t2, :],
                            xh[:, m, jj * 128 : (jj + 1) * 128],
                            qt[:, m, :],
                            start=(m == 0),
                            stop=(m == 1),
                        )
                t8 = sb.tile([128, 2, 512], f8, name=f"t8_{p}_{h}", tag=f"t8_{p}")
                nc.scalar.activation(t8[:, :, :], SP[:, :, :], Tanh, scale=8.0)
                return t8

            def ot_contrib(v8p, OMB, p, t8s):
                # each m-half of OMB is its own accumulation group
                for m in range(2):
                    nc.tensor.matmul(
                        OMB[:, m * 512 : (m + 1) * 512],
                        v8p[p][:, :, m * 128 : (m + 1) * 128],
                        t8s[p][:, :, :],
                        start=(p == 0),
                        stop=(p == 3),
                        perf_mode=DR,
                    )

            def csum_add(h, OMB, dest, split):
                # ot = OMB + colsum (per-partition scalar), f32r.
                # split: column chunks per m-half (subtile unlock of oproj).
                w = 512 // split
                for c in range(split):
                    sl = slice(c * w, (c + 1) * w)
                    nc.vector.tensor_scalar_add(
                        dest[0][:, sl], OMB[:, 0:512][:, sl],
                        csum[:, 2 * h : 2 * h + 1],
                    )
                    nc.scalar.activation(
                        dest[1][:, sl], OMB[:, 512:1024][:, sl], Ident,
                        bias=csum[:, 2 * h + 1 : 2 * h + 2],
                    )

            def oproj(h, ot, wo):
                for mq in range(4):
                    po = PO[mq // 2][:, (mq % 2) * 256 : (mq % 2 + 1) * 256]
                    for m in range(2):
                        mm = nc.tensor.matmul(
                            po,
                            ot[m][:, mq * 128 : (mq + 1) * 128],
                            wo[:, m, :],
                            start=False,
                            stop=False,
                            skip_group_check=True,
                        )
                        if h == 0:
                            add_dep_helper(
                                mm.ins, po_msets[mq // 2].ins,
                                reason="po zeroed before accumulation",
                            )

            def tail_epilogue(ot7, wo):
                # oproj(h7, stop) -> relu -> DMA, pipelined per q-chunk
                dma_eng = [nc.sync, nc.gpsimd, nc.scalar, nc.sync]
                for mq in range(4):
                    po = PO[mq // 2][:, (mq % 2) * 256 : (mq % 2 + 1) * 256]
                    for m in range(2):
                        nc.tensor.matmul(
                            po,
                            ot7[m][:, mq * 128 : (mq + 1) * 128],
                            wo[:, m, :],
                            start=False,
                            stop=(m == 1),
                            skip_group_check=True,
                        )
                    o = osb.tile([128, F], f32, name=f"outsb{mq}", tag=f"outsb{mq}")
                    if mq % 2 == 0:
                        nc.scalar.activation(o[:], po, Relu)
                    else:
                        nc.vector.tensor_scalar_max(o[:], po, 0.0)
                    dma_eng[mq].dma_start(
                        out=out_d[mq * 128 : (mq + 1) * 128, :], in_=o[:]
                    )

            # ---------------- main pipeline ----------------
            ws = [alloc_weights(0), alloc_weights(1)] + [None] * (H - 2)
            prologue_dmas(ws[0], ws[1])
            qt_cur = qm_phase(0, ws[0], qsplit=2)
            v8_cur = v_phase(0, ws[0])

            for h in range(H):
                last = h == H - 1
                if h + 2 < H:
                    ws[h + 2] = dma_weights(h + 2)

                t8s = [scores(h, qt_cur, p) for p in range(4)]
                OMB = psB.tile([128, 1024], f32, name=f"OMB{h}", tag="B")
                ot_contrib(v8_cur, OMB, 0, t8s)
                ot_contrib(v8_cur, OMB, 1, t8s)
                if not last:
                    qt_nxt = qm_phase(h + 1, ws[h + 1])
                ot_contrib(v8_cur, OMB, 2, t8s)
                ot_contrib(v8_cur, OMB, 3, t8s)

                ot = [
                    sb.tile([128, SQ], rdt, name=f"ot{m}_{h}", tag=f"ot{m}")
                    for m in range(2)
                ]
                csum_add(h, OMB, ot, split=(2 if last else 1))

                if not last:
                    v8_nxt = v_phase(h + 1, ws[h + 1])
                    oproj(h, ot, ws[h]["wo"])
                    qt_cur, v8_cur = qt_nxt, v8_nxt
                else:
                    tail_epilogue(ot, ws[h]["wo"])

    nc.finalize()
    return nc


def _get_nc():
    if "nc" not in _CACHE:
        _CACHE["nc"] = _build_nc()
    return _CACHE["nc"]


def _prep_weights(Wq, Wk, Wv, Wo):
    # [F, F*H] with column f_out*H+h  ->  per-head [f_in, f_out]
    wqh = Wq.reshape(F, F, H).transpose(2, 0, 1)  # [H, f_in, f_out]
    wkh = Wk.reshape(F, F, H).transpose(2, 0, 1)
    wvh = np.ascontiguousarray(Wv.reshape(F, F, H).transpose(2, 0, 1))
    # M_h = Wq_h @ Wk_h^T : [H, f_in_q, f_in_k]
    M = np.matmul(np.ascontiguousarray(wqh), wkh.transpose(0, 2, 1))
    # [H, 256, 256] -> [H, 128, 2, 256] (partition-major chunk interleave)
    def lay(w):
        return np.ascontiguousarray(
            w.reshape(H, 2, 128, F).transpose(0, 2, 1, 3)
        )
    # [F*H, F] with row f*H+h -> [H, F, F]; fold the 0.5 centering factor
    woh = Wo.reshape(F, H, F).transpose(1, 0, 2) * 0.5
    return lay(M), lay(wvh), lay(woh), wvh


def kernel(q_input, Wq, Wk, Wv, Wo, _trace=False):
    from concourse.bass_utils import run_bass_kernel_spmd

    nc = _get_nc()
    wm, wv, wo, wvh = _prep_weights(
        np.asarray(Wq, np.float32),
        np.asarray(Wk, np.float32),
        np.asarray(Wv, np.float32),
        np.asarray(Wo, np.float32),
    )
    q_input = np.asarray(q_input, np.float32)

    in_maps = []
    for c in range(N_CORES):
        b, half = c // 2, c % 2
        xT = q_input[b].T  # [F, S]

        def lay_x(cols):
            return np.ascontiguousarray(
                cols.reshape(2, 128, SQ).transpose(1, 0, 2)
            )
        xqT = lay_x(xT[:, half * SQ : (half + 1) * SQ])
        xoT = lay_x(xT[:, (1 - half) * SQ : (2 - half) * SQ])
        # colsum_h = (sum_j x[j,:]) @ Wv_h ; layout [128, h*2+m]
        xsum = q_input[b].sum(axis=0)  # [F]
        cs = (xsum @ wvh).reshape(H, 2, 128)
        csum = np.ascontiguousarray(cs.transpose(2, 0, 1).reshape(128, 2 * H))
        in_maps.append(
            {
                "xqT": xqT,
                "xoT": xoT,
                "Wm": wm,
                "Wv": wv,
                "Wo": wo,
                "csum": csum,
            }
        )

    res = run_bass_kernel_spmd(nc, in_maps, list(range(N_CORES)), trace=_trace)

    out = np.empty((B, S, F), np.float32)
    for c in range(N_CORES):
        b, half = c // 2, c % 2
        out[b, half * SQ : (half + 1) * SQ, :] = res.results[c]["out"]
    if _trace:
        return out, res
    return out
